# revision 15
# baseline (speedup 1.0000x reference)
"""Trainium2 Bass kernel for nn_LlamaQAttention2 (AWQ int4 QKV+O projections,
RoPE, causal attention). 8-core head-parallel tensor parallelism; host sums
the 8 partial outputs (the o_proj all-reduce).

Design (v3):
  - AWQ dequant done HOST-side (numpy); W_qkv / W_o uploaded as f16.
    Removes the on-chip dequant prologue (~100us of vector work) and the
    25MB of zeros/scales broadcast DMA re-reads.
  - X^T prepared HOST-side as f16 [H, BS]; no on-chip transposes or
    f32->f16 converts, and half the X DMA volume.
  - QKV phase k-outer: for each 256-token chunk, loop k-blocks outer and
    m-tiles inner, accumulating 8 open PSUM groups. PE starts as soon as
    the first W tile + X strip land (~2us into the kernel).
  - q^T/k^T (rope fused on PSUM evacuation, pairs of heads per op) and v
    (natural layout) stay SBUF-resident; no DRAM roundtrip.
  - Attention per (batch, head): all score matmuls emitted first, softmax
    (max/exp/sum/normalize) overlaps on vector+scalar, then probs
    transposes, then PV; keeps the PE queue dense so HAM stays warm.
  - O projection in natural layout (lhsT = attnT tile, rhs = Wo rows):
    psum [tokens, 512-col slices]; batch-0 O-proj interleaved between
    batch-1 attention heads; output written f16 via bitcast view.

Self-contained: hardcodes shapes for B=2, S=1024, H=4096, 32 heads.
"""

import math
import numpy as np
from contextlib import ExitStack

import concourse.bass as bass
import concourse.tile as tile
from concourse import bacc, mybir
from concourse.bass_utils import run_bass_kernel_spmd
from concourse.masks import make_identity

F32, F16 = mybir.dt.float32, mybir.dt.float16
I32, I16 = mybir.dt.int32, mybir.dt.int16
A = mybir.AluOpType
ACTF = mybir.ActivationFunctionType

B, S, H = 2, 1024, 4096
BS = B * S                      # 2048 flattened tokens
NH, HD = 32, 128                # heads, head dim
NC = 8                          # cores
HPC = NH // NC                  # 4 heads per core
QC = HPC * HD                   # 512 q (=k=v) columns per core
SCALING = HD ** -0.5
TWO_PI = 2.0 * math.pi
LN1E4 = math.log(10000.0)
AWQ_ORDER = (0, 4, 1, 5, 2, 6, 3, 7)

CH = 256                        # token chunk for qkv
NCH = BS // CH                  # 8 chunks
KB = H // 128                   # 32 contraction blocks
NT = S // 128                   # 8 q/k tiles per batch

_BUILT = None


def _emit(nc, aps):
    pos_ap = aps["pos"]
    xT_ap = aps["xT"]            # [H, BS] f16 (host-transposed)
    w_ap = aps["w"]              # [H, 3*QC] f16 (host-dequanted, q|k|v cols)
    wo_ap = aps["wo"]            # [QC, H] f16 (host-dequanted)
    out_ap = aps["out"].bitcast(F16)   # f16 view of [BS, H//2] f32

    with ExitStack() as ctx:
        tc = aps["tc"]

        const = ctx.enter_context(tc.tile_pool(name="const", bufs=1))
        dram = ctx.enter_context(tc.tile_pool(name="dram", bufs=1, space="DRAM"))

        # identity (for probs transposes) + additive causal mask
        # (natural [q, k] diag block: -60000 where k > q, i.e. col > row)
        ident16 = const.tile([128, 128], F16)
        make_identity(nc, ident16)
        addmask16 = const.tile([128, 128], F16)
        nc.vector.memset(addmask16[:], 0.0)
        nc.gpsimd.affine_select(
            out=addmask16[:], in_=addmask16[:], compare_op=A.is_ge, fill=-60000.0,
            base=0, pattern=[[-1, 128]], channel_multiplier=1)

        # rope cos/sin tables [128, BS] f16 (partition = head-dim % 64 freq)
        cos2 = const.tile([128, BS], F16)
        sin2 = const.tile([128, BS], F16)
        posd = dram.tile([1, BS], F32)

        def emit_angles():
            with tc.tile_pool(name="angprep", bufs=1) as ap_pool:
                pidx = ap_pool.tile([128, 1], I32)
                nc.gpsimd.iota(pidx[:], pattern=[[0, 1]], base=0,
                               channel_multiplier=1)
                nc.vector.tensor_scalar(pidx[:], pidx[:], 63, None,
                                        A.bitwise_and)
                pf = ap_pool.tile([128, 1], F32)
                nc.vector.tensor_copy(pf[:], pidx[:])
                invfreq = ap_pool.tile([128, 1], F32)
                nc.scalar.activation(invfreq[:], pf[:], ACTF.Exp,
                                     scale=-LN1E4 / 64.0)

                HB = BS // 4
                posflat = pos_ap.rearrange("a b -> (a b)")
                for hb in range(4):
                    posi = ap_pool.tile([1, 2 * HB], I32, tag="posi",
                                        name="posi")
                    nc.sync.dma_start(
                        posi[:], posflat[2 * hb * HB:2 * (hb + 1) * HB][None, :])
                    posf = ap_pool.tile([1, HB], F32, tag="posf", name="posf")
                    nc.vector.tensor_copy(posf[:], posi[:, ::2])
                    nc.sync.dma_start(posd[:, hb * HB:(hb + 1) * HB], posf[:])
                    cc = slice(hb * HB, (hb + 1) * HB)
                    posb = ap_pool.tile([128, HB], F32, tag="posb", name="posb")
                    nc.sync.dma_start(posb[:],
                                      posd[:, cc].to_broadcast([128, HB]))
                    ang = ap_pool.tile([128, HB], F32, tag="ang", name="ang")
                    nc.vector.tensor_scalar(ang[:], posb[:], invfreq[:],
                                            1.0 / TWO_PI, A.mult, A.mult)
                    ftmp = ap_pool.tile([128, HB], F32, tag="ftmp", name="ftmp")
                    itmp = ap_pool.tile([128, HB], I32, tag="itmp", name="itmp")
                    gtmp = ap_pool.tile([128, HB], F32, tag="gtmp", name="gtmp")

                    def range_reduce_sin(dst, f_ap):
                        nc.vector.tensor_copy(itmp[:], f_ap)
                        nc.vector.tensor_copy(gtmp[:], itmp[:])
                        nc.vector.tensor_tensor(gtmp[:], f_ap, gtmp[:],
                                                A.subtract)
                        nc.vector.tensor_scalar(ftmp[:], gtmp[:], 0.5, None,
                                                A.is_gt)
                        nc.vector.tensor_tensor(gtmp[:], gtmp[:], ftmp[:],
                                                A.subtract)
                        nc.scalar.activation(dst, gtmp[:], ACTF.Sin,
                                             scale=TWO_PI)

                    range_reduce_sin(sin2[:, cc], ang[:])
                    nc.vector.tensor_scalar(ang[:], ang[:], 0.25, None, A.add)
                    range_reduce_sin(cos2[:, cc], ang[:])
                    nc.vector.tensor_scalar(sin2[0:64, cc], sin2[0:64, cc],
                                            -1.0, None, A.mult)

        # ------------------------------------------------------------------
        # QKV phase: stream W tiles + X^T strips, k-outer matmuls,
        # rope fused on psum evacuation; q^T/k^T/v stay in SBUF
        # ------------------------------------------------------------------
        qkT = ctx.enter_context(tc.tile_pool(name="qkT", bufs=1))
        vstore = ctx.enter_context(tc.tile_pool(name="vstore", bufs=1))
        # q^T, k^T as [128, (head, token)] single tiles
        qT = qkT.tile([128, HPC * BS], F16)
        kT = qkT.tile([128, HPC * BS], F16)
        qTv = qT[:].rearrange("p (h t) -> p h t", t=BS)
        kTv = kT[:].rearrange("p (h t) -> p h t", t=BS)
        vtiles = [vstore.tile([128, QC], F16, tag=f"v{i}", name=f"v{i}")
                  for i in range(BS // 128)]

        emit_angles()

        with ExitStack() as p1:
            wq = p1.enter_context(tc.tile_pool(name="wq", bufs=1))
            xs = p1.enter_context(tc.tile_pool(name="xs", bufs=2))
            rp = p1.enter_context(tc.tile_pool(name="rp", bufs=2))
            psqk = p1.enter_context(tc.tile_pool(name="psqk", bufs=1,
                                                 space="PSUM"))
            psv = p1.enter_context(tc.tile_pool(name="psv", bufs=2,
                                                space="PSUM"))

            # W tiles: emitted first on the sync DMA queue, in k order
            w16 = []
            for k in range(KB):
                w = wq.tile([128, 3 * QC], F16, tag=f"w{k}", name=f"w{k}")
                nc.sync.dma_start(w[:], w_ap[k * 128:(k + 1) * 128, :])
                w16.append(w)

            # chunk-0 X^T strips on the gpsimd DMA queue (parallel with W)
            strips = {}

            def load_strip(c, k):
                st = xs.tile([128, CH], F16, tag=f"xs{k}", name=f"xs{k}")
                nc.gpsimd.dma_start(
                    st[:], xT_ap[k * 128:(k + 1) * 128,
                                 c * CH:(c + 1) * CH])
                strips[(c, k)] = st

            for k in range(KB):
                load_strip(0, k)

            for c in range(NCH):
                ccols = slice(c * CH, (c + 1) * CH)
                pst = [psqk.tile([128, 512], F32, tag=f"qk{j}", name=f"qk{j}")
                       for j in range(4)]
                for k in range(KB):
                    if c + 1 < NCH:
                        load_strip(c + 1, k)
                    st = strips[(c, k)]
                    for m in range(8):
                        # start=True clears has_written for the WHOLE bank:
                        # only the tile's first matmul may set it; the odd
                        # head's k=0 matmul overwrites via the cleared bits.
                        nc.tensor.matmul(
                            pst[m // 2][:, (m % 2) * CH:(m % 2 + 1) * CH],
                            w16[k][:, m * 128:(m + 1) * 128], st[:],
                            start=(k == 0 and m % 2 == 0),
                            stop=(k == KB - 1),
                            skip_group_check=True)
                # rope evacuation: psum tile j holds head pair (2j, 2j+1);
                # doubled tables so each op covers both heads at once
                cosd = rp.tile([128, 2 * CH], F16, tag="cosd", name="cosd")
                sind = rp.tile([128, 2 * CH], F16, tag="sind", name="sind")
                nc.gpsimd.tensor_copy(cosd[:, 0:CH], cos2[:, ccols])
                nc.gpsimd.tensor_copy(cosd[:, CH:2 * CH], cos2[:, ccols])
                nc.gpsimd.tensor_copy(sind[:, 0:CH], sin2[:, ccols])
                nc.gpsimd.tensor_copy(sind[:, CH:2 * CH], sin2[:, ccols])
                for j in range(4):
                    dv = qTv if j < 2 else kTv
                    h0 = (2 * j) % 4
                    dst = dv[:, h0:h0 + 2, ccols]
                    c1 = rp.tile([128, 2 * CH], F16, tag="c1", name="c1")
                    t2 = rp.tile([128, 2 * CH], F16, tag="t2", name="t2")
                    nc.vector.tensor_tensor(c1[:], pst[j][:], cosd[:], A.mult)
                    nc.vector.tensor_tensor(t2[0:64, :], pst[j][64:128, :],
                                            sind[0:64, :], A.mult)
                    nc.vector.tensor_tensor(t2[64:128, :], pst[j][0:64, :],
                                            sind[64:128, :], A.mult)
                    nc.vector.tensor_tensor(dst, c1[:], t2[:], A.add)
                # v natural: lhsT = strip t-slice, rhs = W v-section
                for vt in range(2):
                    pv = psv.tile([128, QC], F32, tag="psv", name="psv")
                    for k in range(KB):
                        nc.tensor.matmul(
                            pv[:], strips[(c, k)][:, vt * 128:(vt + 1) * 128],
                            w16[k][:, 2 * QC:3 * QC],
                            start=(k == 0), stop=(k == KB - 1))
                    nc.scalar.copy(vtiles[2 * c + vt][:], pv[:])
                for k in range(KB):
                    del strips[(c, k)]

        # ------------------------------------------------------------------
        # attention phase + O projection (natural layout)
        # ------------------------------------------------------------------
        with ExitStack() as p2:
            wop = p2.enter_context(tc.tile_pool(name="wo", bufs=1))
            pb = p2.enter_context(tc.tile_pool(name="probs", bufs=1))
            sm = p2.enter_context(tc.tile_pool(name="smtmp", bufs=2))
            at = p2.enter_context(tc.tile_pool(name="attnT", bufs=1))
            ost = p2.enter_context(tc.tile_pool(name="ost", bufs=4))
            ps_sc = p2.enter_context(tc.tile_pool(name="pssc", bufs=4,
                                                  space="PSUM"))
            ps_at = p2.enter_context(tc.tile_pool(name="psat", bufs=1,
                                                  space="PSUM"))
            ps_o = p2.enter_context(tc.tile_pool(name="pso", bufs=2,
                                                 space="PSUM"))

            wo16 = [wop.tile([128, H], F16, tag=f"wo{h}", name=f"wo{h}")
                    for h in range(HPC)]
            for h in range(HPC):
                nc.sync.dma_start(wo16[h][:],
                                  wo_ap[h * 128:(h + 1) * 128, :])

            attnT = [at.tile([128, BS], F16, tag=f"aT{h}", name=f"aT{h}")
                     for h in range(HPC)]

            def emit_oproj(t):
                """O-proj for token tile t: out[t*128:(t+1)*128, :] f16."""
                for n8 in range(H // 512):
                    po = ps_o.tile([128, 512], F32, tag="po", name="po")
                    for h in range(HPC):
                        nc.tensor.matmul(
                            po[:], attnT[h][:, t * 128:(t + 1) * 128],
                            wo16[h][:, n8 * 512:(n8 + 1) * 512],
                            start=(h == 0), stop=(h == HPC - 1))
                    o16 = ost.tile([128, 512], F16, tag="o16", name="o16")
                    if n8 % 2:
                        nc.vector.tensor_copy(o16[:], po[:])
                    else:
                        nc.scalar.copy(o16[:], po[:])
                    nc.sync.dma_start(
                        out_ap[t * 128:(t + 1) * 128,
                               n8 * 512:(n8 + 1) * 512], o16[:])

            def emit_scores(b, h, slot):
                """Score matmuls + softmax for (b, h); returns probs/PT."""
                probs = [pb.tile([128, 128 * (qt + 1)], F16,
                                 tag=f"pn{slot}_{qt}", name=f"pn{qt}")
                         for qt in range(NT)]
                PT = pb.tile([128, NT * S], F16, tag=f"PT{slot}", name="PT")
                for qt in range(NT):
                    ext = 128 * (qt + 1)
                    pcs = []
                    lo = 0
                    while lo < ext:
                        hi = min(lo + 512, ext)
                        scp = ps_sc.tile([128, 512], F32, tag="scps",
                                         name="scps")
                        nc.tensor.matmul(
                            scp[:, 0:hi - lo],
                            qTv[:, h, b * S + qt * 128:b * S + (qt + 1) * 128],
                            kTv[:, h, b * S + lo:b * S + hi],
                            start=True, stop=True)
                        pcs.append((scp, lo, hi))
                        lo = hi
                    scd, dlo, dhi = pcs[-1]
                    nc.vector.tensor_tensor(
                        scd[:, ext - 128 - dlo:ext - dlo],
                        scd[:, ext - 128 - dlo:ext - dlo],
                        addmask16[:], A.add)
                    # row max -> exp bias (-SCALING * max)
                    m = sm.tile([128, 1], F32, tag="rmax", name="rmax")
                    if len(pcs) == 1:
                        nc.vector.tensor_reduce(
                            m[:], pcs[0][0][:, 0:ext],
                            mybir.AxisListType.X, A.max)
                    else:
                        ma = sm.tile([128, 2], F32, tag="ma", name="ma")
                        for pi, (scp, lo, hi) in enumerate(pcs):
                            nc.vector.tensor_reduce(
                                ma[:, pi:pi + 1], scp[:, 0:hi - lo],
                                mybir.AxisListType.X, A.max)
                        nc.vector.tensor_reduce(
                            m[:], ma[:], mybir.AxisListType.X, A.max)
                    negm = sm.tile([128, 1], F32, tag="negm", name="negm")
                    nc.vector.tensor_scalar(negm[:], m[:], -SCALING, None,
                                            A.mult)
                    for (scp, lo, hi) in pcs:
                        nc.scalar.activation(
                            probs[qt][:, lo:hi], scp[:, 0:hi - lo],
                            ACTF.Exp, bias=negm[:], scale=SCALING)
                    rsum = sm.tile([128, 1], F32, tag="rsum", name="rsum")
                    nc.vector.tensor_reduce(rsum[:], probs[qt][:],
                                            mybir.AxisListType.X, A.add)
                    rinv = sm.tile([128, 1], F32, tag="rinv", name="rinv")
                    nc.vector.reciprocal(rinv[:], rsum[:])
                    nc.vector.tensor_scalar(probs[qt][:], probs[qt][:],
                                            rinv[:], None, A.mult)
                return probs, PT

            def emit_tp_pv(b, h, probs, PT):
                """Probs transposes + PV + attnT evac for (b, h)."""
                PTv = PT[:].rearrange("p (s q) -> p s q", q=S)
                at_ps = ps_at.tile([128, S], F32, tag="atps", name="atps")
                for qt in range(NT):
                    for sb0 in range(0, qt + 1, 4):
                        g = min(4, qt + 1 - sb0)
                        tp = ps_sc.tile([128, 512], F16, tag="scps",
                                        name="ptp")
                        for j in range(g):
                            nc.tensor.transpose(
                                tp[:, j * 128:(j + 1) * 128],
                                probs[qt][:, (sb0 + j) * 128:
                                          (sb0 + j + 1) * 128],
                                ident16[:])
                        dstp = PTv[:, sb0:sb0 + g, qt * 128:(qt + 1) * 128]
                        srcp = tp[:, 0:g * 128].rearrange(
                            "p (s q) -> p s q", q=128)
                        if (qt + sb0) % 2:
                            nc.scalar.copy(dstp, srcp)
                        else:
                            nc.vector.tensor_copy(dstp, srcp)
                for sb in range(NT):
                    lo = sb * 128
                    while lo < S:
                        hi = min(lo + 512, S)
                        nc.tensor.matmul(
                            at_ps[:, lo:hi],
                            vtiles[b * 8 + sb][:, h * 128:(h + 1) * 128],
                            PTv[:, sb, lo:hi],
                            start=(sb == 0), stop=(sb == NT - 1),
                            skip_group_check=True)
                        lo = hi
                nc.scalar.copy(attnT[h][:, b * S:(b + 1) * S], at_ps[:])

            # software pipeline: scores(i) overlap transposes/PV(i-1);
            # O-proj(batch 0) fills PE once attnT b0 is complete (after
            # tp_pv(0,3), i.e. from step 5 on)
            seq = [(b, h) for b in range(B) for h in range(HPC)]
            pending = None
            otile = 0
            for i, (b, h) in enumerate(seq):
                sp = emit_scores(b, h, i % 2)
                if pending is not None:
                    pb_, ph_, pprobs, pPT = pending
                    emit_tp_pv(pb_, ph_, pprobs, pPT)
                    if i >= 5:
                        emit_oproj(otile)
                        otile += 1
                        emit_oproj(otile)
                        otile += 1
                pending = (b, h, sp[0], sp[1])
            emit_tp_pv(pending[0], pending[1], pending[2], pending[3])
            while otile < 16:
                emit_oproj(otile)
                otile += 1

            if "dbg_qT" in aps:
                nc.sync.dma_start(aps["dbg_qT"][:], qT[:])
                nc.sync.dma_start(aps["dbg_kT"][:], kT[:])
                for i in range(BS // 128):
                    nc.sync.dma_start(
                        aps["dbg_v"][i * 128:(i + 1) * 128, :], vtiles[i][:])
                for h in range(HPC):
                    nc.sync.dma_start(
                        aps["dbg_aT"][h * 128:(h + 1) * 128, :], attnT[h][:])


def _build(debug_taps=False):
    global _BUILT
    if _BUILT is not None and not debug_taps:
        return _BUILT
    nc = bacc.Bacc("TRN2", target_bir_lowering=False, debug=False,
                   num_devices=NC)
    aps = {
        "pos": nc.dram_tensor("pos", [B, 2 * S], I32,
                              kind="ExternalInput").ap(),
        "xT": nc.dram_tensor("xT", [H, BS], F16, kind="ExternalInput").ap(),
        "w": nc.dram_tensor("w", [H, 3 * QC], F16,
                            kind="ExternalInput").ap(),
        "wo": nc.dram_tensor("wo", [QC, H], F16, kind="ExternalInput").ap(),
        "out": nc.dram_tensor("out", [BS, H // 2], F32,
                              kind="ExternalOutput").ap(),
    }
    if debug_taps:
        aps["dbg_qT"] = nc.dram_tensor("dbg_qT", [128, HPC * BS], F16,
                                       kind="ExternalOutput").ap()
        aps["dbg_kT"] = nc.dram_tensor("dbg_kT", [128, HPC * BS], F16,
                                       kind="ExternalOutput").ap()
        aps["dbg_v"] = nc.dram_tensor("dbg_v", [BS, QC], F16,
                                      kind="ExternalOutput").ap()
        aps["dbg_aT"] = nc.dram_tensor("dbg_aT", [QC, BS], F16,
                                       kind="ExternalOutput").ap()
    with tile.TileContext(nc) as tc:
        aps["tc"] = tc
        _emit(nc, aps)
    nc.compile()
    if not debug_taps:
        _BUILT = nc
    return nc


_SHIFTS_NP = np.array([4 * o for o in AWQ_ORDER], dtype=np.int32)


def _deq_np(qw, qz, sc, c0, c1):
    """AWQ dequant of unpacked column range [c0, c1) -> f16 [rows, c1-c0]."""
    w = np.asarray(qw)[:, c0 // 8:c1 // 8]
    z = np.asarray(qz)[:, c0 // 8:c1 // 8]
    nib = ((w[:, :, None] >> _SHIFTS_NP[None, None, :]) & 0xF).astype(
        np.float32).reshape(w.shape[0], -1)
    zz = ((z[:, :, None] >> _SHIFTS_NP[None, None, :]) & 0xF).astype(
        np.float32).reshape(z.shape[0], -1)
    s = np.asarray(sc, dtype=np.float32)[:, c0:c1]
    gidx = np.arange(w.shape[0]) // 128
    return ((nib - zz[gidx]) * s[gidx]).astype(np.float16)


def _in_maps(positions, hidden_states, qkv_qweight, qkv_qzeros, qkv_scales,
             o_qweight, o_qzeros, o_scales):
    pos = np.ascontiguousarray(
        np.asarray(positions, dtype=np.int64)).view(np.int32).reshape(B, 2 * S)
    x = np.asarray(hidden_states, dtype=np.float32).reshape(BS, H)
    xT = np.ascontiguousarray(x.T.astype(np.float16))
    wo_full = _deq_np(o_qweight, o_qzeros, o_scales, 0, H)   # [H, H] f16

    maps = []
    for i in range(NC):
        uc = QC * i
        w_i = np.concatenate(
            [_deq_np(qkv_qweight, qkv_qzeros, qkv_scales, sec * H + uc,
                     sec * H + uc + QC) for sec in range(3)], axis=1)
        maps.append({
            "pos": pos,
            "xT": xT,
            "w": np.ascontiguousarray(w_i),
            "wo": np.ascontiguousarray(wo_full[uc:uc + QC, :]),
        })
    return maps


def kernel(positions, hidden_states, qkv_qweight, qkv_qzeros, qkv_scales,
           o_qweight, o_qzeros, o_scales, _trace=False, **run_kwargs):
    nc = _build()
    maps = _in_maps(positions, hidden_states, qkv_qweight, qkv_qzeros,
                    qkv_scales, o_qweight, o_qzeros, o_scales)
    res = run_bass_kernel_spmd(nc, maps, core_ids=list(range(NC)),
                               trace=_trace, **run_kwargs)
    acc = np.zeros((BS, H), dtype=np.float32)
    for i in range(NC):
        acc += res.results[i]["out"].view(np.float16).astype(np.float32)
    out = acc.reshape(B, S, H)
    if _trace:
        kernel.last_results = res
    return out


# revision 17
# speedup vs baseline: 1.1838x; 1.1838x over previous
"""Trainium2 Bass kernel for nn_LlamaQAttention2 (AWQ int4 QKV+O projections,
RoPE, causal attention). 8-core head-parallel tensor parallelism; host sums
the 8 partial outputs (the o_proj all-reduce).

Design (v3):
  - AWQ dequant done HOST-side (numpy); W_qkv / W_o uploaded as f16.
    Removes the on-chip dequant prologue (~100us of vector work) and the
    25MB of zeros/scales broadcast DMA re-reads.
  - X^T prepared HOST-side as f16 [H, BS]; no on-chip transposes or
    f32->f16 converts, and half the X DMA volume.
  - QKV phase k-outer: for each 256-token chunk, loop k-blocks outer and
    m-tiles inner, accumulating 8 open PSUM groups. PE starts as soon as
    the first W tile + X strip land (~2us into the kernel).
  - q^T/k^T (rope fused on PSUM evacuation, pairs of heads per op) and v
    (natural layout) stay SBUF-resident; no DRAM roundtrip.
  - Attention per (batch, head): all score matmuls emitted first, softmax
    (max/exp/sum/normalize) overlaps on vector+scalar, then probs
    transposes, then PV; keeps the PE queue dense so HAM stays warm.
  - O projection in natural layout (lhsT = attnT tile, rhs = Wo rows):
    psum [tokens, 512-col slices]; batch-0 O-proj interleaved between
    batch-1 attention heads; output written f16 via bitcast view.

Self-contained: hardcodes shapes for B=2, S=1024, H=4096, 32 heads.
"""

import math
import numpy as np
from contextlib import ExitStack

import concourse.bass as bass
import concourse.tile as tile
from concourse import bacc, mybir
from concourse.bass_utils import run_bass_kernel_spmd
from concourse.masks import make_identity

F32, F16 = mybir.dt.float32, mybir.dt.float16
I32, I16 = mybir.dt.int32, mybir.dt.int16
A = mybir.AluOpType
ACTF = mybir.ActivationFunctionType

B, S, H = 2, 1024, 4096
BS = B * S                      # 2048 flattened tokens
NH, HD = 32, 128                # heads, head dim
NC = 8                          # cores
HPC = NH // NC                  # 4 heads per core
QC = HPC * HD                   # 512 q (=k=v) columns per core
SCALING = HD ** -0.5
TWO_PI = 2.0 * math.pi
LN1E4 = math.log(10000.0)
AWQ_ORDER = (0, 4, 1, 5, 2, 6, 3, 7)

CH = 256                        # token chunk for qkv
NCH = BS // CH                  # 8 chunks
KB = H // 128                   # 32 contraction blocks
NT = S // 128                   # 8 q/k tiles per batch

_BUILT = None


def _emit(nc, aps):
    pos_ap = aps["pos"]
    xT_ap = aps["xT"]            # [H, BS] f16 (host-transposed)
    w_ap = aps["w"]              # [H, 3*QC] f16 (host-dequanted, q|k|v cols)
    wo_ap = aps["wo"]            # [QC, H] f16 (host-dequanted)
    out_ap = aps["out"].bitcast(F16)   # f16 view of [BS, H//2] f32

    with ExitStack() as ctx:
        tc = aps["tc"]

        const = ctx.enter_context(tc.tile_pool(name="const", bufs=1))
        dram = ctx.enter_context(tc.tile_pool(name="dram", bufs=1, space="DRAM"))

        # identity (for probs transposes) + additive causal mask
        # (natural [q, k] diag block: -60000 where k > q, i.e. col > row)
        ident16 = const.tile([128, 128], F16)
        make_identity(nc, ident16)
        addmask16 = const.tile([128, 128], F16)
        nc.vector.memset(addmask16[:], 0.0)
        nc.gpsimd.affine_select(
            out=addmask16[:], in_=addmask16[:], compare_op=A.is_ge, fill=-60000.0,
            base=0, pattern=[[-1, 128]], channel_multiplier=1)

        # rope cos/sin tables [128, BS] f16 (partition = head-dim % 64 freq)
        cos2 = const.tile([128, BS], F16)
        sin2 = const.tile([128, BS], F16)
        posd = dram.tile([1, BS], F32)

        def emit_angles():
            with tc.tile_pool(name="angprep", bufs=1) as ap_pool:
                pidx = ap_pool.tile([128, 1], I32)
                nc.gpsimd.iota(pidx[:], pattern=[[0, 1]], base=0,
                               channel_multiplier=1)
                nc.vector.tensor_scalar(pidx[:], pidx[:], 63, None,
                                        A.bitwise_and)
                pf = ap_pool.tile([128, 1], F32)
                nc.vector.tensor_copy(pf[:], pidx[:])
                invfreq = ap_pool.tile([128, 1], F32)
                nc.scalar.activation(invfreq[:], pf[:], ACTF.Exp,
                                     scale=-LN1E4 / 64.0)

                HB = BS // 4
                posflat = pos_ap.rearrange("a b -> (a b)")
                for hb in range(4):
                    posi = ap_pool.tile([1, 2 * HB], I32, tag="posi",
                                        name="posi")
                    nc.sync.dma_start(
                        posi[:], posflat[2 * hb * HB:2 * (hb + 1) * HB][None, :])
                    posf = ap_pool.tile([1, HB], F32, tag="posf", name="posf")
                    nc.vector.tensor_copy(posf[:], posi[:, ::2])
                    nc.sync.dma_start(posd[:, hb * HB:(hb + 1) * HB], posf[:])
                    cc = slice(hb * HB, (hb + 1) * HB)
                    posb = ap_pool.tile([128, HB], F32, tag="posb", name="posb")
                    nc.sync.dma_start(posb[:],
                                      posd[:, cc].to_broadcast([128, HB]))
                    ang = ap_pool.tile([128, HB], F32, tag="ang", name="ang")
                    nc.vector.tensor_scalar(ang[:], posb[:], invfreq[:],
                                            1.0 / TWO_PI, A.mult, A.mult)
                    ftmp = ap_pool.tile([128, HB], F32, tag="ftmp", name="ftmp")
                    itmp = ap_pool.tile([128, HB], I32, tag="itmp", name="itmp")
                    gtmp = ap_pool.tile([128, HB], F32, tag="gtmp", name="gtmp")

                    def range_reduce_sin(dst, f_ap):
                        nc.vector.tensor_copy(itmp[:], f_ap)
                        nc.vector.tensor_copy(gtmp[:], itmp[:])
                        nc.vector.tensor_tensor(gtmp[:], f_ap, gtmp[:],
                                                A.subtract)
                        nc.vector.tensor_scalar(ftmp[:], gtmp[:], 0.5, None,
                                                A.is_gt)
                        nc.vector.tensor_tensor(gtmp[:], gtmp[:], ftmp[:],
                                                A.subtract)
                        nc.scalar.activation(dst, gtmp[:], ACTF.Sin,
                                             scale=TWO_PI)

                    range_reduce_sin(sin2[:, cc], ang[:])
                    nc.vector.tensor_scalar(ang[:], ang[:], 0.25, None, A.add)
                    range_reduce_sin(cos2[:, cc], ang[:])
                    nc.vector.tensor_scalar(sin2[0:64, cc], sin2[0:64, cc],
                                            -1.0, None, A.mult)

        # ------------------------------------------------------------------
        # QKV phase: stream W tiles + X^T strips, k-outer matmuls,
        # rope fused on psum evacuation; q^T/k^T/v stay in SBUF
        # ------------------------------------------------------------------
        qkT = ctx.enter_context(tc.tile_pool(name="qkT", bufs=1))
        vstore = ctx.enter_context(tc.tile_pool(name="vstore", bufs=1))
        # q^T, k^T as [128, (head, token)] single tiles
        qT = qkT.tile([128, HPC * BS], F16)
        kT = qkT.tile([128, HPC * BS], F16)
        qTv = qT[:].rearrange("p (h t) -> p h t", t=BS)
        kTv = kT[:].rearrange("p (h t) -> p h t", t=BS)
        vtiles = [vstore.tile([128, QC], F16, tag=f"v{i}", name=f"v{i}")
                  for i in range(BS // 128)]

        emit_angles()

        with ExitStack() as p1:
            wq = p1.enter_context(tc.tile_pool(name="wq", bufs=1))
            xs = p1.enter_context(tc.tile_pool(name="xs", bufs=2))
            rp = p1.enter_context(tc.tile_pool(name="rp", bufs=2))
            psqk = p1.enter_context(tc.tile_pool(name="psqk", bufs=1,
                                                 space="PSUM"))
            psv = p1.enter_context(tc.tile_pool(name="psv", bufs=2,
                                                space="PSUM"))

            # W tiles: emitted first on the sync DMA queue, in k order
            w16 = []
            for k in range(KB):
                w = wq.tile([128, 3 * QC], F16, tag=f"w{k}", name=f"w{k}")
                nc.sync.dma_start(w[:], w_ap[k * 128:(k + 1) * 128, :])
                w16.append(w)

            # chunk-0 X^T strips on the gpsimd DMA queue (parallel with W)
            strips = {}

            def load_strip(c, k):
                st = xs.tile([128, CH], F16, tag=f"xs{k}", name=f"xs{k}")
                nc.gpsimd.dma_start(
                    st[:], xT_ap[k * 128:(k + 1) * 128,
                                 c * CH:(c + 1) * CH])
                strips[(c, k)] = st

            for k in range(KB):
                load_strip(0, k)

            for c in range(NCH):
                ccols = slice(c * CH, (c + 1) * CH)
                pst = [psqk.tile([128, 512], F32, tag=f"qk{j}", name=f"qk{j}")
                       for j in range(4)]
                for k in range(KB):
                    if c + 1 < NCH:
                        load_strip(c + 1, k)
                    st = strips[(c, k)]
                    for m in range(8):
                        # start=True clears has_written for the WHOLE bank:
                        # only the tile's first matmul may set it; the odd
                        # head's k=0 matmul overwrites via the cleared bits.
                        nc.tensor.matmul(
                            pst[m // 2][:, (m % 2) * CH:(m % 2 + 1) * CH],
                            w16[k][:, m * 128:(m + 1) * 128], st[:],
                            start=(k == 0 and m % 2 == 0),
                            stop=(k == KB - 1),
                            skip_group_check=True)
                # rope evacuation: psum tile j holds head pair (2j, 2j+1);
                # doubled tables so each op covers both heads at once
                cosd = rp.tile([128, 2 * CH], F16, tag="cosd", name="cosd")
                sind = rp.tile([128, 2 * CH], F16, tag="sind", name="sind")
                nc.gpsimd.tensor_copy(cosd[:, 0:CH], cos2[:, ccols])
                nc.gpsimd.tensor_copy(cosd[:, CH:2 * CH], cos2[:, ccols])
                nc.gpsimd.tensor_copy(sind[:, 0:CH], sin2[:, ccols])
                nc.gpsimd.tensor_copy(sind[:, CH:2 * CH], sin2[:, ccols])
                for j in range(4):
                    dv = qTv if j < 2 else kTv
                    h0 = (2 * j) % 4
                    dst = dv[:, h0:h0 + 2, ccols]
                    c1 = rp.tile([128, 2 * CH], F16, tag="c1", name="c1")
                    t2 = rp.tile([128, 2 * CH], F16, tag="t2", name="t2")
                    nc.vector.tensor_tensor(c1[:], pst[j][:], cosd[:], A.mult)
                    nc.vector.tensor_tensor(t2[0:64, :], pst[j][64:128, :],
                                            sind[0:64, :], A.mult)
                    nc.vector.tensor_tensor(t2[64:128, :], pst[j][0:64, :],
                                            sind[64:128, :], A.mult)
                    nc.vector.tensor_tensor(dst, c1[:], t2[:], A.add)
                # v natural: lhsT = strip t-slice, rhs = W v-section
                for vt in range(2):
                    pv = psv.tile([128, QC], F32, tag="psv", name="psv")
                    for k in range(KB):
                        nc.tensor.matmul(
                            pv[:], strips[(c, k)][:, vt * 128:(vt + 1) * 128],
                            w16[k][:, 2 * QC:3 * QC],
                            start=(k == 0), stop=(k == KB - 1))
                    nc.scalar.copy(vtiles[2 * c + vt][:], pv[:])
                for k in range(KB):
                    del strips[(c, k)]

        # ------------------------------------------------------------------
        # attention phase + O projection (natural layout)
        # ------------------------------------------------------------------
        with ExitStack() as p2:
            wop = p2.enter_context(tc.tile_pool(name="wo", bufs=1))
            pb = p2.enter_context(tc.tile_pool(name="probs", bufs=1))
            sm = p2.enter_context(tc.tile_pool(name="smtmp", bufs=2))
            at = p2.enter_context(tc.tile_pool(name="attnT", bufs=1))
            ost = p2.enter_context(tc.tile_pool(name="ost", bufs=4))
            ps_sc = p2.enter_context(tc.tile_pool(name="pssc", bufs=4,
                                                  space="PSUM"))
            ps_at = p2.enter_context(tc.tile_pool(name="psat", bufs=1,
                                                  space="PSUM"))
            ps_o = p2.enter_context(tc.tile_pool(name="pso", bufs=2,
                                                 space="PSUM"))

            wo16 = [wop.tile([128, H], F16, tag=f"wo{h}", name=f"wo{h}")
                    for h in range(HPC)]
            for h in range(HPC):
                nc.sync.dma_start(wo16[h][:],
                                  wo_ap[h * 128:(h + 1) * 128, :])

            attnT = [at.tile([128, BS], F16, tag=f"aT{h}", name=f"aT{h}")
                     for h in range(HPC)]

            def emit_oproj(t):
                """O-proj for token tile t: out[t*128:(t+1)*128, :] f16."""
                for n8 in range(H // 512):
                    po = ps_o.tile([128, 512], F32, tag="po", name="po")
                    for h in range(HPC):
                        nc.tensor.matmul(
                            po[:], attnT[h][:, t * 128:(t + 1) * 128],
                            wo16[h][:, n8 * 512:(n8 + 1) * 512],
                            start=(h == 0), stop=(h == HPC - 1))
                    o16 = ost.tile([128, 512], F16, tag="o16", name="o16")
                    if n8 % 2:
                        nc.vector.tensor_copy(o16[:], po[:])
                    else:
                        nc.scalar.copy(o16[:], po[:])
                    nc.sync.dma_start(
                        out_ap[t * 128:(t + 1) * 128,
                               n8 * 512:(n8 + 1) * 512], o16[:])

            def emit_scores(b, h, slot):
                """Score matmuls + softmax for (b, h); returns probs/PT."""
                probs = [pb.tile([128, 128 * (qt + 1)], F16,
                                 tag=f"pn{slot}_{qt}", name=f"pn{qt}")
                         for qt in range(NT)]
                PT = pb.tile([128, NT * S], F16, tag=f"PT{slot}", name="PT")
                for qt in range(NT):
                    ext = 128 * (qt + 1)
                    pcs = []
                    lo = 0
                    while lo < ext:
                        hi = min(lo + 512, ext)
                        scp = ps_sc.tile([128, 512], F32, tag="scps",
                                         name="scps")
                        nc.tensor.matmul(
                            scp[:, 0:hi - lo],
                            qTv[:, h, b * S + qt * 128:b * S + (qt + 1) * 128],
                            kTv[:, h, b * S + lo:b * S + hi],
                            start=True, stop=True)
                        pcs.append((scp, lo, hi))
                        lo = hi
                    # causal mask on the diagonal block: accumulate I^T @ M
                    # into the scores psum (PE, start=False) — keeps the
                    # vector engine out of the masking business
                    scd, dlo, dhi = pcs[-1]
                    nc.tensor.matmul(
                        scd[:, ext - 128 - dlo:ext - dlo], ident16[:],
                        addmask16[:], start=False, stop=True,
                        skip_group_check=True)
                    # row max -> exp bias (-SCALING * max)
                    m = sm.tile([128, 1], F32, tag="rmax", name="rmax")
                    if len(pcs) == 1:
                        nc.vector.tensor_reduce(
                            m[:], pcs[0][0][:, 0:ext],
                            mybir.AxisListType.X, A.max)
                    else:
                        ma = sm.tile([128, 2], F32, tag="ma", name="ma")
                        for pi, (scp, lo, hi) in enumerate(pcs):
                            nc.vector.tensor_reduce(
                                ma[:, pi:pi + 1], scp[:, 0:hi - lo],
                                mybir.AxisListType.X, A.max)
                        nc.vector.tensor_reduce(
                            m[:], ma[:], mybir.AxisListType.X, A.max)
                    negm = sm.tile([128, 1], F32, tag="negm", name="negm")
                    nc.vector.tensor_scalar(negm[:], m[:], -SCALING, None,
                                            A.mult)
                    for (scp, lo, hi) in pcs:
                        nc.scalar.activation(
                            probs[qt][:, lo:hi], scp[:, 0:hi - lo],
                            ACTF.Exp, bias=negm[:], scale=SCALING)
                    rsum = sm.tile([128, 1], F32, tag="rsum", name="rsum")
                    nc.gpsimd.tensor_reduce(rsum[:], probs[qt][:],
                                            mybir.AxisListType.X, A.add)
                    rinv = sm.tile([128, 1], F32, tag="rinv", name="rinv")
                    nc.vector.reciprocal(rinv[:], rsum[:])
                    nc.gpsimd.tensor_scalar(probs[qt][:], probs[qt][:],
                                            rinv[:], None, A.mult)
                return probs, PT

            def emit_tp_pv(b, h, probs, PT):
                """Probs transposes + PV + attnT evac for (b, h)."""
                PTv = PT[:].rearrange("p (s q) -> p s q", q=S)
                at_ps = ps_at.tile([128, S], F32, tag="atps", name="atps")
                for qt in range(NT):
                    for sb0 in range(0, qt + 1, 4):
                        g = min(4, qt + 1 - sb0)
                        tp = ps_sc.tile([128, 512], F16, tag="scps",
                                        name="ptp")
                        for j in range(g):
                            nc.tensor.transpose(
                                tp[:, j * 128:(j + 1) * 128],
                                probs[qt][:, (sb0 + j) * 128:
                                          (sb0 + j + 1) * 128],
                                ident16[:])
                        dstp = PTv[:, sb0:sb0 + g, qt * 128:(qt + 1) * 128]
                        srcp = tp[:, 0:g * 128].rearrange(
                            "p (s q) -> p s q", q=128)
                        if (qt + sb0) % 2:
                            nc.scalar.copy(dstp, srcp)
                        else:
                            nc.vector.tensor_copy(dstp, srcp)
                for sb in range(NT):
                    lo = sb * 128
                    while lo < S:
                        hi = min(lo + 512, S)
                        nc.tensor.matmul(
                            at_ps[:, lo:hi],
                            vtiles[b * 8 + sb][:, h * 128:(h + 1) * 128],
                            PTv[:, sb, lo:hi],
                            start=(sb == 0), stop=(sb == NT - 1),
                            skip_group_check=True)
                        lo = hi
                nc.scalar.copy(attnT[h][:, b * S:(b + 1) * S], at_ps[:])

            # software pipeline: scores(i) overlap transposes/PV(i-1);
            # O-proj(batch 0) fills PE once attnT b0 is complete (after
            # tp_pv(0,3), i.e. from step 5 on)
            seq = [(b, h) for b in range(B) for h in range(HPC)]
            pending = None
            otile = 0
            for i, (b, h) in enumerate(seq):
                sp = emit_scores(b, h, i % 2)
                if pending is not None:
                    pb_, ph_, pprobs, pPT = pending
                    emit_tp_pv(pb_, ph_, pprobs, pPT)
                    if i >= 5:
                        emit_oproj(otile)
                        otile += 1
                        emit_oproj(otile)
                        otile += 1
                pending = (b, h, sp[0], sp[1])
            emit_tp_pv(pending[0], pending[1], pending[2], pending[3])
            while otile < 16:
                emit_oproj(otile)
                otile += 1

            if "dbg_qT" in aps:
                nc.sync.dma_start(aps["dbg_qT"][:], qT[:])
                nc.sync.dma_start(aps["dbg_kT"][:], kT[:])
                for i in range(BS // 128):
                    nc.sync.dma_start(
                        aps["dbg_v"][i * 128:(i + 1) * 128, :], vtiles[i][:])
                for h in range(HPC):
                    nc.sync.dma_start(
                        aps["dbg_aT"][h * 128:(h + 1) * 128, :], attnT[h][:])


def _build(debug_taps=False):
    global _BUILT
    if _BUILT is not None and not debug_taps:
        return _BUILT
    nc = bacc.Bacc("TRN2", target_bir_lowering=False, debug=False,
                   num_devices=NC)
    aps = {
        "pos": nc.dram_tensor("pos", [B, 2 * S], I32,
                              kind="ExternalInput").ap(),
        "xT": nc.dram_tensor("xT", [H, BS], F16, kind="ExternalInput").ap(),
        "w": nc.dram_tensor("w", [H, 3 * QC], F16,
                            kind="ExternalInput").ap(),
        "wo": nc.dram_tensor("wo", [QC, H], F16, kind="ExternalInput").ap(),
        "out": nc.dram_tensor("out", [BS, H // 2], F32,
                              kind="ExternalOutput").ap(),
    }
    if debug_taps:
        aps["dbg_qT"] = nc.dram_tensor("dbg_qT", [128, HPC * BS], F16,
                                       kind="ExternalOutput").ap()
        aps["dbg_kT"] = nc.dram_tensor("dbg_kT", [128, HPC * BS], F16,
                                       kind="ExternalOutput").ap()
        aps["dbg_v"] = nc.dram_tensor("dbg_v", [BS, QC], F16,
                                      kind="ExternalOutput").ap()
        aps["dbg_aT"] = nc.dram_tensor("dbg_aT", [QC, BS], F16,
                                       kind="ExternalOutput").ap()
    with tile.TileContext(nc) as tc:
        aps["tc"] = tc
        _emit(nc, aps)
    nc.compile()
    if not debug_taps:
        _BUILT = nc
    return nc


_SHIFTS_NP = np.array([4 * o for o in AWQ_ORDER], dtype=np.int32)


def _deq_np(qw, qz, sc, c0, c1):
    """AWQ dequant of unpacked column range [c0, c1) -> f16 [rows, c1-c0]."""
    w = np.asarray(qw)[:, c0 // 8:c1 // 8]
    z = np.asarray(qz)[:, c0 // 8:c1 // 8]
    nib = ((w[:, :, None] >> _SHIFTS_NP[None, None, :]) & 0xF).astype(
        np.float32).reshape(w.shape[0], -1)
    zz = ((z[:, :, None] >> _SHIFTS_NP[None, None, :]) & 0xF).astype(
        np.float32).reshape(z.shape[0], -1)
    s = np.asarray(sc, dtype=np.float32)[:, c0:c1]
    gidx = np.arange(w.shape[0]) // 128
    return ((nib - zz[gidx]) * s[gidx]).astype(np.float16)


def _in_maps(positions, hidden_states, qkv_qweight, qkv_qzeros, qkv_scales,
             o_qweight, o_qzeros, o_scales):
    pos = np.ascontiguousarray(
        np.asarray(positions, dtype=np.int64)).view(np.int32).reshape(B, 2 * S)
    x = np.asarray(hidden_states, dtype=np.float32).reshape(BS, H)
    xT = np.ascontiguousarray(x.T.astype(np.float16))
    wo_full = _deq_np(o_qweight, o_qzeros, o_scales, 0, H)   # [H, H] f16

    maps = []
    for i in range(NC):
        uc = QC * i
        w_i = np.concatenate(
            [_deq_np(qkv_qweight, qkv_qzeros, qkv_scales, sec * H + uc,
                     sec * H + uc + QC) for sec in range(3)], axis=1)
        maps.append({
            "pos": pos,
            "xT": xT,
            "w": np.ascontiguousarray(w_i),
            "wo": np.ascontiguousarray(wo_full[uc:uc + QC, :]),
        })
    return maps


def kernel(positions, hidden_states, qkv_qweight, qkv_qzeros, qkv_scales,
           o_qweight, o_qzeros, o_scales, _trace=False, **run_kwargs):
    nc = _build()
    maps = _in_maps(positions, hidden_states, qkv_qweight, qkv_qzeros,
                    qkv_scales, o_qweight, o_qzeros, o_scales)
    res = run_bass_kernel_spmd(nc, maps, core_ids=list(range(NC)),
                               trace=_trace, **run_kwargs)
    acc = np.zeros((BS, H), dtype=np.float32)
    for i in range(NC):
        acc += res.results[i]["out"].view(np.float16).astype(np.float32)
    out = acc.reshape(B, S, H)
    if _trace:
        kernel.last_results = res
    return out
